# revision 1
# baseline (speedup 1.0000x reference)
"""Trainium2 Bass kernel for the ESM contrastive projection head loss.

Problem (hardcoded): x [512, 512, 960] f32; two 2-layer MLPs (codon for batch
rows 0:256, amino for 256:512) applied to mean-pooled x; pairwise cosine
similarity of the concatenated projections z [512, 240]; diag-masked,
temperature-scaled InfoNCE-style NLL, mean over rows.

Strategy: data-parallel over batch across 8 NeuronCores (64 rows each).
Each core streams its 126 MB x-shard (DMA-bound, ~358 GB/s/core), mean-pools
via DVE chunk-adds + a one-hot-window fp32 matmul for the partition-dim
reduction, runs its half's MLP, allgathers z [64,240] -> [512,240] in-kernel,
computes its 64 rows of the similarity/logsumexp, and outputs nll [64,1].
Host sums and divides.
"""
import contextlib
import ctypes
import os
import sys
import types

import numpy as np

B = 512
S = 512
D = 960
NCORES = 8
BPC = B // NCORES           # 64 batch rows per core
SLAB_B = 2                  # batch rows per DMA slab
NSLAB = BPC // SLAB_B       # 32
INV_T = 10.0                # 1 / temperature
NEG_T = -655040.0           # NEG_INF / temperature
EPS = 1e-8
D1 = D // 2                 # 480
D2 = D // 4                 # 240

_CACHE = {}
LAST_RESULT = None
TRACE_CORES = [0]


def _install_ntff_hook():
    """Make run_bass_kernel_spmd(trace=True) work under axon (test.py only)."""
    if "antenv.axon_hooks" in sys.modules:
        return
    so_path = "/opt/axon/libaxon_pjrt.so"
    try:
        lib = ctypes.CDLL(so_path)
    except OSError:
        return
    if not hasattr(lib, "axon_start_nrt_profile"):
        return
    lib.axon_start_nrt_profile.argtypes = [ctypes.POINTER(ctypes.c_int64), ctypes.c_size_t]
    lib.axon_start_nrt_profile.restype = ctypes.c_int64
    lib.axon_stop_nrt_profile.argtypes = [ctypes.c_char_p]
    lib.axon_stop_nrt_profile.restype = ctypes.c_int64

    @contextlib.contextmanager
    def _hook(output_dir, device_ids):
        import jax
        jax.devices()
        if device_ids:
            ids = (ctypes.c_int64 * len(device_ids))(*device_ids)
            rc = lib.axon_start_nrt_profile(ids, len(device_ids))
        else:
            rc = lib.axon_start_nrt_profile(None, 0)
        if rc != 0:
            raise RuntimeError(f"axon_start_nrt_profile rc={rc}")
        try:
            yield
        finally:
            n = lib.axon_stop_nrt_profile(str(output_dir).encode())
            print(f"profile: {n} file(s) written to {output_dir}", file=sys.stderr)

    mod = types.ModuleType("antenv.axon_hooks")
    mod.get_axon_ntff_profile_hook = lambda: _hook
    mod.set_axon_ntff_profile_hook = lambda h: None
    sys.modules["antenv.axon_hooks"] = mod


def _build_nc():
    import concourse.tile as tile
    from concourse import bacc, mybir

    f32 = mybir.dt.float32
    add = mybir.AluOpType.add
    mult = mybir.AluOpType.mult
    sub = mybir.AluOpType.subtract
    amax = mybir.AluOpType.max
    AF = mybir.ActivationFunctionType
    AX = mybir.AxisListType

    nc = bacc.Bacc("TRN2", target_bir_lowering=False, debug=False,
                   enable_asserts=True, num_devices=NCORES)

    xs = nc.dram_tensor("xs", [BPC, S, D], f32, kind="ExternalInput").ap()
    w1 = nc.dram_tensor("w1", [D, D1], f32, kind="ExternalInput").ap()
    b1 = nc.dram_tensor("b1", [D1], f32, kind="ExternalInput").ap()
    w2 = nc.dram_tensor("w2", [D1, D2], f32, kind="ExternalInput").ap()
    b2 = nc.dram_tensor("b2", [D2], f32, kind="ExternalInput").ap()
    jwin = nc.dram_tensor("jwin", [128, 128], f32, kind="ExternalInput").ap()
    ident = nc.dram_tensor("ident", [128, 128], f32, kind="ExternalInput").ap()
    mulm = nc.dram_tensor("mulm", [BPC, B], f32, kind="ExternalInput").ap()
    addm = nc.dram_tensor("addm", [BPC, B], f32, kind="ExternalInput").ap()
    posm = nc.dram_tensor("posm", [BPC, B], f32, kind="ExternalInput").ap()
    out = nc.dram_tensor("nll", [BPC, 1], f32, kind="ExternalOutput").ap()

    with tile.TileContext(nc) as tc:
        with contextlib.ExitStack() as ctx:
            ep = ctx.enter_context
            consts = ep(tc.tile_pool(name="consts", bufs=1))
            xpool = ep(tc.tile_pool(name="xslab", bufs=3))
            apool = ep(tc.tile_pool(name="acc", bufs=3))
            spool = ep(tc.tile_pool(name="small", bufs=1))
            scr = ep(tc.tile_pool(name="scratch", bufs=1))
            dram = ep(tc.tile_pool(name="dram", bufs=1, space="DRAM"))

            # --- constant loads (ACT HWDGE ring; x-slabs use the SP ring) ---
            jwin_sb = consts.tile([128, 128], f32, tag="jwin")
            nc.scalar.dma_start(jwin_sb[:], jwin)
            ident_sb = consts.tile([128, 128], f32, tag="ident")
            nc.scalar.dma_start(ident_sb[:], ident)
            w1_sb = consts.tile([120, 8, D1], f32, tag="w1")
            nc.scalar.dma_start(w1_sb[:], w1.rearrange("(k p) j -> p k j", p=120))
            w2_sb = consts.tile([120, 4, D2], f32, tag="w2")
            nc.scalar.dma_start(w2_sb[:], w2.rearrange("(k p) j -> p k j", p=120))
            b1_sb = consts.tile([120, 4], f32, tag="b1")
            nc.scalar.dma_start(b1_sb[:], b1.rearrange("(g p) -> p g", p=120))
            b2_sb = consts.tile([120, 2], f32, tag="b2")
            nc.scalar.dma_start(b2_sb[:], b2.rearrange("(g p) -> p g", p=120))
            mulm_sb = consts.tile([BPC, B], f32, tag="mulm")
            nc.scalar.dma_start(mulm_sb[:], mulm)
            addm_sb = consts.tile([BPC, B], f32, tag="addm")
            nc.scalar.dma_start(addm_sb[:], addm)
            posm_sb = consts.tile([BPC, B], f32, tag="posm")
            nc.scalar.dma_start(posm_sb[:], posm)

            # --- warm up the collective path early (junk payload, result
            # folded into the output with x0 so it is not dead-code) ---
            # scalar ring: the wg readback waits on the collective and must
            # not sit in the sync FIFO ahead of the x-slab DMAs
            wb = dram.tile([BPC, 8], f32, tag="wb")
            wg = dram.tile([B, 8], f32, tag="wg")
            nc.gpsimd.dma_start(wb[:], jwin_sb[0:BPC, 0:8])
            nc.gpsimd.collective_compute(
                "AllGather", mybir.AluOpType.bypass,
                replica_groups=[list(range(NCORES))],
                ins=[wb.opt()], outs=[wg.opt()],
            )
            wg_sb = spool.tile([BPC, 1], f32, tag="wg")
            nc.gpsimd.dma_start(wg_sb[:], wg[0:BPC, 0:1])


            ones_sb = consts.tile([120, 1], f32, tag="ones")
            nc.vector.memset(ones_sb[:], 1.0)
            zeros_sb = consts.tile([120, BPC], f32, tag="zeros")
            nc.vector.memset(zeros_sb[:], 0.0)

            # --- phase A: stream x, mean-pool into PSUM [64, 960] ---
            # (jwin holds 1/512 so the matmul accumulates the mean directly)
            pooled_sb = spool.tile([BPC, D], f32, tag="pooled")
            with tc.tile_pool(name="pps", bufs=1, space="PSUM") as pps:
                pooled_a = pps.tile([BPC, 512], f32, tag="pa")
                pooled_b = pps.tile([BPC, D - 512], f32, tag="pb")
                for i in range(NSLAB):
                    # p-major layout: partition p holds 8 consecutive rows of
                    # the 2-batch slab -> fully contiguous per-partition DMA
                    slab = xpool.tile([128, 8, D], f32, tag="slab")
                    src = xs[SLAB_B * i:SLAB_B * (i + 1)].rearrange(
                        "b (q m) d -> (b q) m d", m=8)
                    nc.sync.dma_start(slab[:], src)
                    nc.vector.tensor_tensor(slab[:, 0:4, :], slab[:, 0:4, :],
                                            slab[:, 4:8, :], add)
                    nc.vector.tensor_tensor(slab[:, 0:2, :], slab[:, 0:2, :],
                                            slab[:, 2:4, :], add)
                    acc = apool.tile([128, D], f32, tag="acc")
                    nc.vector.tensor_tensor(acc[:], slab[:, 0, :], slab[:, 1, :], add)
                    jw = jwin_sb[:, 64 - SLAB_B * i:128 - SLAB_B * i]
                    nc.tensor.matmul(pooled_a[:], jw, acc[:, 0:512],
                                     start=(i == 0), stop=(i == NSLAB - 1))
                    nc.tensor.matmul(pooled_b[:], jw, acc[:, 512:D],
                                     start=(i == 0), stop=(i == NSLAB - 1))

                nc.vector.tensor_copy(pooled_sb[:, 0:512], pooled_a[:])
                nc.vector.tensor_copy(pooled_sb[:, 512:D], pooled_b[:])

            ps2 = ep(tc.tile_pool(name="ps2", bufs=2, space="PSUM"))
            psmm = ep(tc.tile_pool(name="psmm", bufs=4, space="PSUM"))
            ps1 = ep(tc.tile_pool(name="ps1", bufs=1, space="PSUM"))

            # --- pooled^T [120, 8, 64] via PE transposes ---
            pT_sb = spool.tile([120, 8, BPC], f32, tag="pT")
            for g in range(8):
                tp = ps2.tile([120, BPC], f32, tag="small")
                nc.tensor.transpose(tp[:], pooled_sb[:, 120 * g:120 * (g + 1)],
                                    ident_sb[0:BPC, 0:BPC])
                nc.vector.tensor_copy(pT_sb[:, g, :], tp[:])

            # --- MLP layer 1: h^T [120, 4, 64] = relu(W1^T pooled^T + b1) ---
            h_sb = spool.tile([120, 4, BPC], f32, tag="h")
            for jg in range(4):
                hp = psmm.tile([120, BPC], f32, tag="mm")
                for kc in range(8):
                    nc.tensor.matmul(hp[:], w1_sb[:, kc, 120 * jg:120 * (jg + 1)],
                                     pT_sb[:, kc, :], start=(kc == 0), stop=(kc == 7))
                nc.vector.scalar_tensor_tensor(h_sb[:, jg, :], hp[:],
                                               b1_sb[:, jg:jg + 1], zeros_sb[:],
                                               add, amax)

            # --- MLP layer 2: z^T [120, 2, 64] = W2^T h^T + b2 ---
            zT_sb = spool.tile([120, 2, BPC], f32, tag="zT")
            for og in range(2):
                zp = psmm.tile([120, BPC], f32, tag="mm")
                for kc in range(4):
                    nc.tensor.matmul(zp[:], w2_sb[:, kc, 120 * og:120 * (og + 1)],
                                     h_sb[:, kc, :], start=(kc == 0), stop=(kc == 3))
                nc.vector.tensor_scalar_add(zT_sb[:, og, :], zp[:],
                                            b2_sb[:, og:og + 1])

            # --- local norms [1, 64] + inverses via ones-matmul ---
            lsq = scr.tile([120, 2, BPC], f32, tag="lsq")
            nc.vector.tensor_tensor(lsq[:], zT_sb[:], zT_sb[:], mult)
            nlq_ps = ps2.tile([1, BPC], f32, tag="small")
            nc.tensor.matmul(nlq_ps[:], ones_sb[:], lsq[:, 0, :],
                             start=True, stop=False)
            nc.tensor.matmul(nlq_ps[:], ones_sb[:], lsq[:, 1, :],
                             start=False, stop=True)
            nlr_sb = spool.tile([1, BPC], f32, tag="nlrsb")
            nc.scalar.sqrt(nlr_sb[:], nlq_ps[:])
            nlr_inv = spool.tile([1, BPC], f32, tag="nlrinv")
            nc.vector.reciprocal(nlr_inv[:], nlr_sb[:])

            # --- allgather payload [256, 64]: rows 0:240 = z^T, row 240 =
            # row norms, rows 241:256 pad.  Gathering z^T directly avoids
            # any post-gather transposes and norm recomputation. ---
            zb = dram.tile([256, BPC], f32, tag="zb")
            zg = dram.tile([256 * NCORES, BPC], f32, tag="zg")
            nc.sync.dma_start(
                zb[0:240, :].rearrange("(og p) b -> p og b", p=120), zT_sb[:])
            nc.sync.dma_start(zb[240:241, :], nlr_inv[:])
            nc.sync.dma_start(zb[241:256, :], zeros_sb[0:15, :])
            nc.gpsimd.collective_compute(
                "AllGather", mybir.AluOpType.bypass,
                replica_groups=[list(range(NCORES))],
                ins=[zb.opt()], outs=[zg.opt()],
            )
            zgv = zg[:].rearrange("(c r) b -> r c b", r=256)
            zfT_sb = spool.tile([120, 2, B], f32, tag="zfT")
            for og in range(2):
                nc.sync.dma_start(
                    zfT_sb[:, og, :].rearrange("p (c b) -> p c b", b=BPC),
                    zgv[120 * og:120 * (og + 1)])
            nrow_inv = spool.tile([1, B], f32, tag="nrowinv")
            nc.sync.dma_start(nrow_inv[:].rearrange("p (c b) -> p c b", b=BPC),
                              zgv[240:241])

            # --- similarity dot and inverse-norm outer product [64, 512] ---
            s_ps = ps1.tile([BPC, B], f32, tag="sp")
            nc.tensor.matmul(s_ps[:], zT_sb[:, 0, :], zfT_sb[:, 0, :],
                             start=True, stop=False)
            nc.tensor.matmul(s_ps[:], zT_sb[:, 1, :], zfT_sb[:, 1, :],
                             start=False, stop=True)
            o_ps = ps1.tile([BPC, B], f32, tag="op")
            nc.tensor.matmul(o_ps[:], nlr_inv[:], nrow_inv[:],
                             start=True, stop=True)

            # --- logits = cos/T with diag replaced; logsumexp; nll ---
            rin = scr.tile([BPC, B], f32, tag="rin")
            nc.vector.tensor_tensor(rin[:], o_ps[:], mulm_sb[:], mult)
            logits = scr.tile([BPC, B], f32, tag="logits")
            nc.vector.tensor_tensor(logits[:], s_ps[:], rin[:], mult)
            nc.vector.tensor_tensor(logits[:], logits[:], addm_sb[:], add)

            pos = spool.tile([BPC, 1], f32, tag="pos")
            e_sb = scr.tile([BPC, B], f32, tag="esb")
            nc.vector.scalar_tensor_tensor(e_sb[:], logits[:], 1.0, posm_sb[:],
                                           mult, mult, accum_out=pos[:])
            mx = spool.tile([BPC, 1], f32, tag="mx")
            nc.vector.tensor_reduce(mx[:], logits[:], AX.X, amax)
            nmx = spool.tile([BPC, 1], f32, tag="nmx")
            nc.vector.tensor_scalar_mul(nmx[:], mx[:], -1.0)
            esum = spool.tile([BPC, 1], f32, tag="esum")
            nc.scalar.activation(e_sb[:], logits[:], AF.Exp,
                                 bias=nmx[:], scale=1.0, accum_out=esum[:])
            lnv = spool.tile([BPC, 1], f32, tag="lnv")
            nc.scalar.activation(lnv[:], esum[:], AF.Ln)
            nll_sb = spool.tile([BPC, 1], f32, tag="nll")
            nc.vector.tensor_tensor(nll_sb[:], lnv[:], mx[:], add)
            nc.vector.tensor_tensor(nll_sb[:], nll_sb[:], pos[:], sub)
            # keep the warmup collective alive: nll += 0 * wg
            nc.vector.scalar_tensor_tensor(nll_sb[:], wg_sb[:], 0.0, nll_sb[:],
                                           mult, add)

            nc.sync.dma_start(out, nll_sb[:])

    nc.compile()
    return nc


def _host_inputs(x, W1c, b1c, W2c, b2c, W1a, b1a, W2a, b2a):
    x = np.ascontiguousarray(np.asarray(x, dtype=np.float32))
    # window matrix: slab i selects columns [64-2i, 128-2i); partition p
    # (batch half p//64) must hit output row 2i + p//64, so the fixed column
    # is 64 + p//64.  Values hold the mean's 1/512.
    jwin = np.zeros((128, 128), dtype=np.float32)
    jwin[np.arange(128), 64 + np.arange(128) // 64] = 1.0 / S
    ident = np.eye(128, dtype=np.float32)
    in_maps = []
    for c in range(NCORES):
        rows = np.arange(BPC)
        gl = 64 * c + rows
        mulm = np.full((BPC, B), INV_T, dtype=np.float32)
        mulm[rows, gl] = 0.0
        addm = np.zeros((BPC, B), dtype=np.float32)
        addm[rows, gl] = NEG_T
        posm = np.zeros((BPC, B), dtype=np.float32)
        posm[rows, (gl + B // 2) % B] = 1.0
        if c < NCORES // 2:
            w1, bb1, w2, bb2 = W1c, b1c, W2c, b2c
        else:
            w1, bb1, w2, bb2 = W1a, b1a, W2a, b2a
        in_maps.append({
            "xs": x[BPC * c:BPC * (c + 1)],
            "w1": np.ascontiguousarray(np.asarray(w1, np.float32)),
            "b1": np.ascontiguousarray(np.asarray(bb1, np.float32)),
            "w2": np.ascontiguousarray(np.asarray(w2, np.float32)),
            "b2": np.ascontiguousarray(np.asarray(bb2, np.float32)),
            "jwin": jwin,
            "ident": ident,
            "mulm": mulm,
            "addm": addm,
            "posm": posm,
        })
    return in_maps


def kernel(x, W1c, b1c, W2c, b2c, W1a, b1a, W2a, b2a):
    global LAST_RESULT
    trace = bool(os.environ.get("BASS_TRACE"))
    if trace:
        _install_ntff_hook()
    from concourse import bass_utils
    if trace:
        bass_utils.upload_artifacts = lambda tmpdir: "local://skipped"

    if "nc" not in _CACHE:
        _CACHE["nc"] = _build_nc()
    nc = _CACHE["nc"]

    in_maps = _host_inputs(x, W1c, b1c, W2c, b2c, W1a, b1a, W2a, b2a)
    kwargs = {}
    if trace:
        kwargs = {"trace": True, "trace_cores": TRACE_CORES}
    res = bass_utils.run_bass_kernel_spmd(
        nc, in_maps, list(range(NCORES)), **kwargs)
    LAST_RESULT = res
    nll = np.concatenate([res.results[c]["nll"][:, 0] for c in range(NCORES)])
    return np.asarray(nll.mean(dtype=np.float64), dtype=np.float32)



# revision 5
# speedup vs baseline: 1.0431x; 1.0431x over previous
"""Trainium2 Bass kernel for the ESM contrastive projection head loss.

Problem (hardcoded): x [512, 512, 960] f32; two 2-layer MLPs (codon for batch
rows 0:256, amino for 256:512) applied to mean-pooled x; pairwise cosine
similarity of the concatenated projections z [512, 240]; diag-masked,
temperature-scaled InfoNCE-style NLL, mean over rows.

Strategy: data-parallel over batch across 8 NeuronCores (64 rows each).
Each core streams its 126 MB x-shard (DMA-bound, ~345 GB/s/core), mean-pools
via DVE chunk-adds + a one-hot-window fp32r matmul for the partition-dim
reduction, runs its half's MLP in bf16, allgathers z^T (bf16) in-kernel,
computes its 64 rows of the similarity/logsumexp, and outputs nll [64,1].
Host sums and divides.

vs baseline: constants packed into 2 contiguous DMAs (was 9 strided ones that
clogged HWDGE semaphore lanes and delayed early x slabs), xpool bufs 4 (was
3), post-pool math in bf16 (single-pass PE matmuls, half-size allgather),
pooling matmul in fp32r (single-pass).
"""
import contextlib
import ctypes
import os
import sys
import types

import numpy as np

B = 512
S = 512
D = 960
NCORES = 8
BPC = B // NCORES           # 64 batch rows per core
SLAB_B = 2                  # batch rows per DMA slab
NSLAB = BPC // SLAB_B       # 32
INV_T = 10.0                # 1 / temperature
NEG_T = -655040.0           # NEG_INF / temperature
EPS = 1e-8
D1 = D // 2                 # 480
D2 = D // 4                 # 240

# packed-constant column offsets (f32 pack)
C4_JW = 0                   # jwin [128, 128]
C4_MUL = 128                # mulm [64, 512]
C4_ADD = 640                # addm [64, 512]
C4_POS = 1152               # posm [64, 512]
C4_B1 = 1664                # b1 grouped [120, 4]
C4_B2 = 1668                # b2 grouped [120, 2]
C4_W = 1670
# bf16 pack
C2_ID = 0                   # identity [128, 128]
C2_W1 = 128                 # w1 grouped [120, 8*480]
C2_W2 = 3968                # w2 grouped [120, 4*240]
C2_W = 4928

_CACHE = {}
LAST_RESULT = None
TRACE_CORES = [0]


def _install_ntff_hook():
    """Make run_bass_kernel_spmd(trace=True) work under axon (test.py only)."""
    if "antenv.axon_hooks" in sys.modules:
        return
    so_path = "/opt/axon/libaxon_pjrt.so"
    try:
        lib = ctypes.CDLL(so_path)
    except OSError:
        return
    if not hasattr(lib, "axon_start_nrt_profile"):
        return
    lib.axon_start_nrt_profile.argtypes = [ctypes.POINTER(ctypes.c_int64), ctypes.c_size_t]
    lib.axon_start_nrt_profile.restype = ctypes.c_int64
    lib.axon_stop_nrt_profile.argtypes = [ctypes.c_char_p]
    lib.axon_stop_nrt_profile.restype = ctypes.c_int64

    @contextlib.contextmanager
    def _hook(output_dir, device_ids):
        import jax
        jax.devices()
        if device_ids:
            ids = (ctypes.c_int64 * len(device_ids))(*device_ids)
            rc = lib.axon_start_nrt_profile(ids, len(device_ids))
        else:
            rc = lib.axon_start_nrt_profile(None, 0)
        if rc != 0:
            raise RuntimeError(f"axon_start_nrt_profile rc={rc}")
        try:
            yield
        finally:
            n = lib.axon_stop_nrt_profile(str(output_dir).encode())
            print(f"profile: {n} file(s) written to {output_dir}", file=sys.stderr)

    mod = types.ModuleType("antenv.axon_hooks")
    mod.get_axon_ntff_profile_hook = lambda: _hook
    mod.set_axon_ntff_profile_hook = lambda h: None
    sys.modules["antenv.axon_hooks"] = mod


def _build_nc():
    import concourse.tile as tile
    from concourse import bacc, mybir

    f32 = mybir.dt.float32
    f32r = mybir.dt.float32r
    bf16 = mybir.dt.bfloat16
    add = mybir.AluOpType.add
    mult = mybir.AluOpType.mult
    sub = mybir.AluOpType.subtract
    amax = mybir.AluOpType.max
    AF = mybir.ActivationFunctionType
    AX = mybir.AxisListType

    nc = bacc.Bacc("TRN2", target_bir_lowering=False, debug=False,
                   enable_asserts=True, num_devices=NCORES)

    xs = nc.dram_tensor("xs", [BPC, S, D], f32, kind="ExternalInput").ap()
    cp4 = nc.dram_tensor("cp4", [128, C4_W], f32, kind="ExternalInput").ap()
    cp2 = nc.dram_tensor("cp2", [128, C2_W], bf16, kind="ExternalInput").ap()
    out = nc.dram_tensor("nll", [BPC, 1], f32, kind="ExternalOutput").ap()

    with tile.TileContext(nc) as tc:
        with contextlib.ExitStack() as ctx:
            ep = ctx.enter_context
            consts = ep(tc.tile_pool(name="consts", bufs=1))
            xpool = ep(tc.tile_pool(name="xslab", bufs=4))
            apool = ep(tc.tile_pool(name="acc", bufs=3))
            spool = ep(tc.tile_pool(name="small", bufs=1))
            scr = ep(tc.tile_pool(name="scratch", bufs=1))
            dram = ep(tc.tile_pool(name="dram", bufs=1, space="DRAM"))

            # --- packed constant loads (ACT HWDGE ring; x uses the SP ring) ---
            c4 = consts.tile([128, C4_W], f32, tag="c4")
            nc.scalar.dma_start(c4[:], cp4)
            c2 = consts.tile([128, C2_W], bf16, tag="c2")
            nc.scalar.dma_start(c2[:], cp2)

            jwin_sb = c4[:, C4_JW:C4_JW + 128]
            mulm_sb = c4[0:BPC, C4_MUL:C4_MUL + B]
            addm_sb = c4[0:BPC, C4_ADD:C4_ADD + B]
            posm_sb = c4[0:BPC, C4_POS:C4_POS + B]
            b1_sb = c4[0:120, C4_B1:C4_B1 + 4]
            b2_sb = c4[0:120, C4_B2:C4_B2 + 2]
            identb = c2[:, C2_ID:C2_ID + 128]
            w1_sb = c2[0:120, C2_W1:C2_W1 + 8 * D1].rearrange(
                "p (k j) -> p k j", k=8)
            w2_sb = c2[0:120, C2_W2:C2_W2 + 4 * D2].rearrange(
                "p (k j) -> p k j", k=4)

            # --- warm up the collective path early (junk payload, result
            # folded into the output with x0 so it is not dead-code) ---
            wb = dram.tile([BPC, 8], f32, tag="wb")
            wg = dram.tile([B, 8], f32, tag="wg")
            nc.gpsimd.dma_start(wb[:], c4[0:BPC, 0:8])
            nc.gpsimd.collective_compute(
                "AllGather", mybir.AluOpType.bypass,
                replica_groups=[list(range(NCORES))],
                ins=[wb.opt()], outs=[wg.opt()],
            )
            wg_sb = spool.tile([BPC, 1], f32, tag="wg")
            nc.gpsimd.dma_start(wg_sb[:], wg[0:BPC, 0:1])

            ones_sb = consts.tile([120, 1], f32, tag="ones")
            nc.vector.memset(ones_sb[:], 1.0)
            zeros_sb = consts.tile([120, BPC], f32, tag="zeros")
            nc.vector.memset(zeros_sb[:], 0.0)
            # fp32r copy of the pooling window (single-pass PE matmuls need
            # fp32r-rounded producers)
            jwr_sb = consts.tile([128, 128], f32r, tag="jwr")
            nc.vector.tensor_copy(jwr_sb[:], jwin_sb)

            # --- phase A: stream x, mean-pool into PSUM [64, 960] ---
            # (jwin holds 1/512 so the matmul accumulates the mean directly;
            # fp32r = single-pass PE mode, plenty for a mean)
            pooled_sb = spool.tile([BPC, D], bf16, tag="pooled")
            with tc.tile_pool(name="pps", bufs=1, space="PSUM") as pps:
                pooled_a = pps.tile([BPC, 512], f32, tag="pa")
                pooled_b = pps.tile([BPC, D - 512], f32, tag="pb")
                for i in range(NSLAB):
                    # p-major layout: partition p holds 8 consecutive rows of
                    # the 2-batch slab -> fully contiguous per-partition DMA
                    slab = xpool.tile([128, 8, D], f32, tag="slab")
                    src = xs[SLAB_B * i:SLAB_B * (i + 1)].rearrange(
                        "b (q m) d -> (b q) m d", m=8)
                    nc.sync.dma_start(slab[:], src)
                    nc.vector.tensor_tensor(slab[:, 0:4, :], slab[:, 0:4, :],
                                            slab[:, 4:8, :], add)
                    nc.vector.tensor_tensor(slab[:, 0:2, :], slab[:, 0:2, :],
                                            slab[:, 2:4, :], add)
                    acc = apool.tile([128, D], f32r, tag="acc")
                    nc.vector.tensor_tensor(acc[:], slab[:, 0, :], slab[:, 1, :], add)
                    jw = jwr_sb[:, 64 - SLAB_B * i:128 - SLAB_B * i]
                    nc.tensor.matmul(pooled_a[:], jw, acc[:, 0:512],
                                     start=(i == 0), stop=(i == NSLAB - 1))
                    nc.tensor.matmul(pooled_b[:], jw, acc[:, 512:D],
                                     start=(i == 0), stop=(i == NSLAB - 1))

                nc.vector.tensor_copy(pooled_sb[:, 0:512], pooled_a[:])
                nc.vector.tensor_copy(pooled_sb[:, 512:D], pooled_b[:])

            ps2 = ep(tc.tile_pool(name="ps2", bufs=2, space="PSUM"))
            psmm = ep(tc.tile_pool(name="psmm", bufs=4, space="PSUM"))
            ps1 = ep(tc.tile_pool(name="ps1", bufs=1, space="PSUM"))

            # --- pooled^T [120, 8, 64] via PE transposes (bf16) ---
            pT_sb = spool.tile([120, 8, BPC], bf16, tag="pT")
            for g in range(8):
                tp = ps2.tile([120, BPC], bf16, tag="small")
                nc.tensor.transpose(tp[:], pooled_sb[:, 120 * g:120 * (g + 1)],
                                    identb[0:BPC, 0:BPC])
                nc.vector.tensor_copy(pT_sb[:, g, :], tp[:])

            # --- MLP layer 1: h^T [120, 4, 64] = relu(W1^T pooled^T + b1) ---
            h_sb = spool.tile([120, 4, BPC], bf16, tag="h")
            for jg in range(4):
                hp = psmm.tile([120, BPC], f32, tag="mm")
                for kc in range(8):
                    nc.tensor.matmul(hp[:], w1_sb[:, kc, 120 * jg:120 * (jg + 1)],
                                     pT_sb[:, kc, :], start=(kc == 0), stop=(kc == 7))
                nc.vector.scalar_tensor_tensor(h_sb[:, jg, :], hp[:],
                                               b1_sb[:, jg:jg + 1], zeros_sb[:],
                                               add, amax)

            # --- MLP layer 2: z^T [120, 2, 64] = W2^T h^T + b2 ---
            zT_sb = spool.tile([120, 2, BPC], bf16, tag="zT")
            for og in range(2):
                zp = psmm.tile([120, BPC], f32, tag="mm")
                for kc in range(4):
                    nc.tensor.matmul(zp[:], w2_sb[:, kc, 120 * og:120 * (og + 1)],
                                     h_sb[:, kc, :], start=(kc == 0), stop=(kc == 3))
                nc.vector.tensor_scalar_add(zT_sb[:, og, :], zp[:],
                                            b2_sb[:, og:og + 1])

            # --- local norms [1, 64] + inverses via ones-matmul ---
            lsq = scr.tile([120, 2, BPC], f32, tag="lsq")
            nc.vector.tensor_tensor(lsq[:], zT_sb[:], zT_sb[:], mult)
            nlq_ps = psmm.tile([1, BPC], f32, tag="mm")
            nc.tensor.matmul(nlq_ps[:], ones_sb[:], lsq[:, 0, :],
                             start=True, stop=False)
            nc.tensor.matmul(nlq_ps[:], ones_sb[:], lsq[:, 1, :],
                             start=False, stop=True)
            nlr_sb = spool.tile([1, BPC], f32, tag="nlrsb")
            nc.scalar.sqrt(nlr_sb[:], nlq_ps[:])
            nlr_inv = spool.tile([1, BPC], f32, tag="nlrinv")
            nc.vector.reciprocal(nlr_inv[:], nlr_sb[:])
            nlr_bf = spool.tile([1, BPC], bf16, tag="nlrbf")
            nc.vector.tensor_copy(nlr_bf[:], nlr_inv[:])

            # --- allgather payload [241, 64] bf16: rows 0:240 = z^T, row 240
            # = inverse row norms.  Gathering z^T directly avoids any
            # post-gather transposes and norm recomputation. ---
            zb = dram.tile([241, BPC], bf16, tag="zb")
            zg = dram.tile([241 * NCORES, BPC], bf16, tag="zg")
            nc.sync.dma_start(
                zb[0:240, :].rearrange("(og p) b -> p og b", p=120), zT_sb[:])
            nc.sync.dma_start(zb[240:241, :], nlr_bf[:])
            nc.gpsimd.collective_compute(
                "AllGather", mybir.AluOpType.bypass,
                replica_groups=[list(range(NCORES))],
                ins=[zb.opt()], outs=[zg.opt()],
            )
            zgv = zg[:].rearrange("(c r) b -> r c b", r=241)
            zfT_sb = spool.tile([120, 2, B], bf16, tag="zfT")
            for og in range(2):
                nc.sync.dma_start(
                    zfT_sb[:, og, :].rearrange("p (c b) -> p c b", b=BPC),
                    zgv[120 * og:120 * (og + 1)])
            nrow_bf = spool.tile([1, B], bf16, tag="nrowbf")
            nc.sync.dma_start(nrow_bf[:].rearrange("p (c b) -> p c b", b=BPC),
                              zgv[240:241])

            # --- similarity dot and inverse-norm outer product [64, 512] ---
            s_ps = ps1.tile([BPC, B], f32, tag="sp")
            nc.tensor.matmul(s_ps[:], zT_sb[:, 0, :], zfT_sb[:, 0, :],
                             start=True, stop=False)
            nc.tensor.matmul(s_ps[:], zT_sb[:, 1, :], zfT_sb[:, 1, :],
                             start=False, stop=True)
            o_ps = ps1.tile([BPC, B], f32, tag="op")
            nc.tensor.matmul(o_ps[:], nlr_bf[:], nrow_bf[:],
                             start=True, stop=True)

            # --- logits = cos/T with diag replaced; logsumexp; nll ---
            rin = scr.tile([BPC, B], f32, tag="rin")
            nc.vector.tensor_tensor(rin[:], o_ps[:], mulm_sb, mult)
            logits = scr.tile([BPC, B], f32, tag="logits")
            nc.vector.tensor_tensor(logits[:], s_ps[:], rin[:], mult)
            nc.vector.tensor_tensor(logits[:], logits[:], addm_sb, add)

            pos = spool.tile([BPC, 1], f32, tag="pos")
            e_sb = scr.tile([BPC, B], f32, tag="esb")
            nc.vector.scalar_tensor_tensor(e_sb[:], logits[:], 1.0, posm_sb,
                                           mult, mult, accum_out=pos[:])
            mx = spool.tile([BPC, 1], f32, tag="mx")
            nc.vector.tensor_reduce(mx[:], logits[:], AX.X, amax)
            nmx = spool.tile([BPC, 1], f32, tag="nmx")
            nc.vector.tensor_scalar_mul(nmx[:], mx[:], -1.0)
            esum = spool.tile([BPC, 1], f32, tag="esum")
            nc.scalar.activation(e_sb[:], logits[:], AF.Exp,
                                 bias=nmx[:], scale=1.0, accum_out=esum[:])
            lnv = spool.tile([BPC, 1], f32, tag="lnv")
            nc.scalar.activation(lnv[:], esum[:], AF.Ln)
            nll_sb = spool.tile([BPC, 1], f32, tag="nll")
            nc.vector.tensor_tensor(nll_sb[:], lnv[:], mx[:], add)
            nc.vector.tensor_tensor(nll_sb[:], nll_sb[:], pos[:], sub)
            # keep the warmup collective alive: nll += 0 * wg
            nc.vector.scalar_tensor_tensor(nll_sb[:], wg_sb[:], 0.0, nll_sb[:],
                                           mult, add)

            nc.sync.dma_start(out, nll_sb[:])

    nc.compile()
    return nc


def _bf16(a):
    import ml_dtypes
    return np.asarray(a, dtype=np.float32).astype(ml_dtypes.bfloat16)


def _host_inputs(x, W1c, b1c, W2c, b2c, W1a, b1a, W2a, b2a):
    x = np.ascontiguousarray(np.asarray(x, dtype=np.float32))
    # window matrix: slab i selects columns [64-2i, 128-2i); partition p
    # (batch half p//64) must hit output row 2i + p//64, so the fixed column
    # is 64 + p//64.  Values hold the mean's 1/512.
    jwin = np.zeros((128, 128), dtype=np.float32)
    jwin[np.arange(128), 64 + np.arange(128) // 64] = 1.0 / S
    in_maps = []
    for c in range(NCORES):
        rows = np.arange(BPC)
        gl = BPC * c + rows
        if c < NCORES // 2:
            w1, bb1, w2, bb2 = W1c, b1c, W2c, b2c
        else:
            w1, bb1, w2, bb2 = W1a, b1a, W2a, b2a
        w1 = np.asarray(w1, np.float32)
        w2 = np.asarray(w2, np.float32)

        cp4 = np.zeros((128, C4_W), dtype=np.float32)
        cp4[:, C4_JW:C4_JW + 128] = jwin
        mulm = np.full((BPC, B), INV_T, dtype=np.float32)
        mulm[rows, gl] = 0.0
        cp4[0:BPC, C4_MUL:C4_MUL + B] = mulm
        addm = np.zeros((BPC, B), dtype=np.float32)
        addm[rows, gl] = NEG_T
        cp4[0:BPC, C4_ADD:C4_ADD + B] = addm
        posm = np.zeros((BPC, B), dtype=np.float32)
        posm[rows, (gl + B // 2) % B] = 1.0
        cp4[0:BPC, C4_POS:C4_POS + B] = posm
        cp4[0:120, C4_B1:C4_B1 + 4] = np.asarray(bb1, np.float32).reshape(4, 120).T
        cp4[0:120, C4_B2:C4_B2 + 2] = np.asarray(bb2, np.float32).reshape(2, 120).T

        cp2 = np.zeros((128, C2_W), dtype=np.float32)
        cp2[:, C2_ID:C2_ID + 128] = np.eye(128, dtype=np.float32)
        # w1 grouped: [p, k*480 + j] = W1[k*120 + p, j]
        cp2[0:120, C2_W1:C2_W1 + 8 * D1] = (
            w1.reshape(8, 120, D1).transpose(1, 0, 2).reshape(120, 8 * D1))
        cp2[0:120, C2_W2:C2_W2 + 4 * D2] = (
            w2.reshape(4, 120, D2).transpose(1, 0, 2).reshape(120, 4 * D2))

        in_maps.append({
            "xs": x[BPC * c:BPC * (c + 1)],
            "cp4": cp4,
            "cp2": _bf16(cp2),
        })
    return in_maps


def kernel(x, W1c, b1c, W2c, b2c, W1a, b1a, W2a, b2a):
    global LAST_RESULT
    trace = bool(os.environ.get("BASS_TRACE"))
    if trace:
        _install_ntff_hook()
    from concourse import bass_utils
    if trace:
        bass_utils.upload_artifacts = lambda tmpdir: "local://skipped"

    if "nc" not in _CACHE:
        _CACHE["nc"] = _build_nc()
    nc = _CACHE["nc"]

    in_maps = _host_inputs(x, W1c, b1c, W2c, b2c, W1a, b1a, W2a, b2a)
    kwargs = {}
    if trace:
        kwargs = {"trace": True, "trace_cores": TRACE_CORES}
    res = bass_utils.run_bass_kernel_spmd(
        nc, in_maps, list(range(NCORES)), **kwargs)
    LAST_RESULT = res
    nll = np.concatenate([res.results[c]["nll"][:, 0] for c in range(NCORES)])
    return np.asarray(nll.mean(dtype=np.float64), dtype=np.float32)


# revision 10
# speedup vs baseline: 1.0829x; 1.0382x over previous
"""Trainium2 Bass kernel for the ESM contrastive projection head loss.

Problem (hardcoded): x [512, 512, 960] f32; two 2-layer MLPs (codon for batch
rows 0:256, amino for 256:512) applied to mean-pooled x; pairwise cosine
similarity of the concatenated projections z [512, 240]; diag-masked,
temperature-scaled InfoNCE-style NLL, mean over rows.

Strategy: data-parallel over batch across 8 NeuronCores (64 rows each).
Each core streams its 126 MB x-shard (DMA-bound, ~345 GB/s/core), mean-pools
via DVE chunk-adds + a one-hot-window fp32r matmul for the partition-dim
reduction, runs its half's MLP in bf16, allgathers z^T (bf16) in-kernel,
computes its 64 rows of the similarity/logsumexp, and outputs nll [64,1].
Host sums and divides.

vs baseline: constants packed into 2 contiguous DMAs (was 9 strided ones that
clogged HWDGE semaphore lanes and delayed early x slabs), xpool bufs 4 (was
3), post-pool math in bf16 (single-pass PE matmuls, half-size allgather),
pooling matmul in fp32r (single-pass).
"""
import contextlib
import ctypes
import os
import sys
import types

import numpy as np

B = 512
S = 512
D = 960
NCORES = 8
BPC = B // NCORES           # 64 batch rows per core
SLAB_B = 2                  # batch rows per DMA slab
NSLAB = BPC // SLAB_B       # 32
INV_T = 10.0                # 1 / temperature
NEG_T = -655040.0           # NEG_INF / temperature
EPS = 1e-8
D1 = D // 2                 # 480
D2 = D // 4                 # 240

# packed-constant column offsets (f32 pack)
C4_JW = 0                   # jwin [128, 128]
C4_DIA = 128                # diagm (1.0 on own-diagonal cols) [64, 512]
C4_POS = 640                # posm [64, 512]
C4_B1 = 1152                # b1 grouped [120, 4]
C4_B2 = 1156                # b2 grouped [120, 2]
C4_W = 1158
# bf16 pack
C2_ID = 0                   # identity [128, 128]
C2_W1 = 128                 # w1 grouped [120, 8*480]
C2_W2 = 3968                # w2 grouped [120, 4*240]
C2_W = 4928

_CACHE = {}
LAST_RESULT = None
TRACE_CORES = [0]


def _install_ntff_hook():
    """Make run_bass_kernel_spmd(trace=True) work under axon (test.py only)."""
    if "antenv.axon_hooks" in sys.modules:
        return
    so_path = "/opt/axon/libaxon_pjrt.so"
    try:
        lib = ctypes.CDLL(so_path)
    except OSError:
        return
    if not hasattr(lib, "axon_start_nrt_profile"):
        return
    lib.axon_start_nrt_profile.argtypes = [ctypes.POINTER(ctypes.c_int64), ctypes.c_size_t]
    lib.axon_start_nrt_profile.restype = ctypes.c_int64
    lib.axon_stop_nrt_profile.argtypes = [ctypes.c_char_p]
    lib.axon_stop_nrt_profile.restype = ctypes.c_int64

    @contextlib.contextmanager
    def _hook(output_dir, device_ids):
        import jax
        jax.devices()
        if device_ids:
            ids = (ctypes.c_int64 * len(device_ids))(*device_ids)
            rc = lib.axon_start_nrt_profile(ids, len(device_ids))
        else:
            rc = lib.axon_start_nrt_profile(None, 0)
        if rc != 0:
            raise RuntimeError(f"axon_start_nrt_profile rc={rc}")
        try:
            yield
        finally:
            n = lib.axon_stop_nrt_profile(str(output_dir).encode())
            print(f"profile: {n} file(s) written to {output_dir}", file=sys.stderr)

    mod = types.ModuleType("antenv.axon_hooks")
    mod.get_axon_ntff_profile_hook = lambda: _hook
    mod.set_axon_ntff_profile_hook = lambda h: None
    sys.modules["antenv.axon_hooks"] = mod


def _build_nc():
    import concourse.tile as tile
    from concourse import bacc, mybir

    f32 = mybir.dt.float32
    f32r = mybir.dt.float32r
    bf16 = mybir.dt.bfloat16
    add = mybir.AluOpType.add
    mult = mybir.AluOpType.mult
    sub = mybir.AluOpType.subtract
    amax = mybir.AluOpType.max
    AF = mybir.ActivationFunctionType
    AX = mybir.AxisListType

    nc = bacc.Bacc("TRN2", target_bir_lowering=False, debug=False,
                   enable_asserts=True, num_devices=NCORES)

    xs = nc.dram_tensor("xs", [BPC, S, D], f32, kind="ExternalInput").ap()
    cp4 = nc.dram_tensor("cp4", [128, C4_W], f32, kind="ExternalInput").ap()
    cp2 = nc.dram_tensor("cp2", [128, C2_W], bf16, kind="ExternalInput").ap()
    out = nc.dram_tensor("nll", [BPC, 3], f32, kind="ExternalOutput").ap()

    with tile.TileContext(nc) as tc:
        with contextlib.ExitStack() as ctx:
            ep = ctx.enter_context
            consts = ep(tc.tile_pool(name="consts", bufs=1))
            xpool = ep(tc.tile_pool(name="xslab", bufs=4))
            apool = ep(tc.tile_pool(name="acc", bufs=3))
            spool = ep(tc.tile_pool(name="small", bufs=1))
            scr = ep(tc.tile_pool(name="scratch", bufs=1))
            dram = ep(tc.tile_pool(name="dram", bufs=1, space="DRAM"))

            # --- packed constant loads (ACT HWDGE ring; x uses the SP ring) ---
            c4 = consts.tile([128, C4_W], f32, tag="c4")
            nc.scalar.dma_start(c4[:], cp4)
            c2 = consts.tile([128, C2_W], bf16, tag="c2")
            nc.scalar.dma_start(c2[:], cp2)

            jwin_sb = c4[:, C4_JW:C4_JW + 128]
            diagm_sb = c4[0:BPC, C4_DIA:C4_DIA + B]
            posm_sb = c4[0:BPC, C4_POS:C4_POS + B]
            b1_sb = c4[0:120, C4_B1:C4_B1 + 4]
            b2_sb = c4[0:120, C4_B2:C4_B2 + 2]
            identb = c2[:, C2_ID:C2_ID + 128]
            w1_sb = c2[0:120, C2_W1:C2_W1 + 8 * D1].rearrange(
                "p (k j) -> p k j", k=8)
            w2_sb = c2[0:120, C2_W2:C2_W2 + 4 * D2].rearrange(
                "p (k j) -> p k j", k=4)

            # --- warm up the collective path early (junk payload, result
            # folded into the output with x0 so it is not dead-code) ---
            wb = dram.tile([BPC, 8], f32, tag="wb")
            wg = dram.tile([B, 8], f32, tag="wg")
            nc.gpsimd.dma_start(wb[:], c4[0:BPC, 0:8])
            nc.gpsimd.collective_compute(
                "AllGather", mybir.AluOpType.bypass,
                replica_groups=[list(range(NCORES))],
                ins=[wb.opt()], outs=[wg.opt()],
            )
            wg_sb = spool.tile([BPC, 1], f32, tag="wg")
            nc.gpsimd.dma_start(wg_sb[:], wg[0:BPC, 0:1])

            ones_sb = consts.tile([120, 1], bf16, tag="ones")
            nc.vector.memset(ones_sb[:], 1.0)
            zeros_sb = consts.tile([120, BPC], f32, tag="zeros")
            nc.vector.memset(zeros_sb[:], 0.0)
            # fp32r copy of the pooling window (single-pass PE matmuls need
            # fp32r-rounded producers)
            jwr_sb = consts.tile([128, 128], f32r, tag="jwr")
            nc.vector.tensor_copy(jwr_sb[:], jwin_sb)

            # --- phase A: stream x, mean-pool into PSUM [64, 960] ---
            # (jwin holds 1/512 so the matmul accumulates the mean directly;
            # fp32r = single-pass PE mode, plenty for a mean)
            pooled_sb = spool.tile([BPC, D], bf16, tag="pooled")
            with tc.tile_pool(name="pps", bufs=1, space="PSUM") as pps:
                phalf0 = pps.tile([BPC, 480], f32, tag="p0")
                phalf1 = pps.tile([BPC, 480], f32, tag="p1")
                phalf = [phalf0, phalf1]
                for i in range(NSLAB):
                    # p-major layout: partition p holds 8 consecutive rows of
                    # the 2-batch slab -> fully contiguous per-partition DMA
                    slab = xpool.tile([128, 8, D], f32, tag="slab")
                    src = xs[SLAB_B * i:SLAB_B * (i + 1)].rearrange(
                        "b (q m) d -> (b q) m d", m=8)
                    nc.sync.dma_start(slab[:], src)
                    acc = apool.tile([128, D], f32r, tag="acc")
                    jw = jwr_sb[:, 64 - SLAB_B * i:128 - SLAB_B * i]
                    # D-halves pipeline independently so the last slab's
                    # serial add chain is half as long before the matmul
                    for h in range(2):
                        sl = slice(480 * h, 480 * (h + 1))
                        nc.vector.tensor_tensor(slab[:, 0:4, sl], slab[:, 0:4, sl],
                                                slab[:, 4:8, sl], add)
                        nc.vector.tensor_tensor(slab[:, 0:2, sl], slab[:, 0:2, sl],
                                                slab[:, 2:4, sl], add)
                        nc.vector.tensor_tensor(acc[:, sl], slab[:, 0, sl],
                                                slab[:, 1, sl], add)
                        nc.tensor.matmul(phalf[h][:], jw, acc[:, sl],
                                         start=(i == 0), stop=(i == NSLAB - 1))

                for h in range(2):
                    nc.vector.tensor_copy(pooled_sb[:, 480 * h:480 * (h + 1)],
                                          phalf[h][:])

            ps2 = ep(tc.tile_pool(name="ps2", bufs=2, space="PSUM"))
            psmm = ep(tc.tile_pool(name="psmm", bufs=4, space="PSUM"))
            ps1 = ep(tc.tile_pool(name="ps1", bufs=1, space="PSUM"))

            # --- pooled^T [120, 8, 64] via PE transposes (bf16) ---
            pT_sb = spool.tile([120, 8, BPC], bf16, tag="pT")
            for g in range(8):
                tp = ps2.tile([120, BPC], bf16, tag="small")
                nc.tensor.transpose(tp[:], pooled_sb[:, 120 * g:120 * (g + 1)],
                                    identb[0:BPC, 0:BPC])
                nc.vector.tensor_copy(pT_sb[:, g, :], tp[:])

            # --- MLP layer 1: h^T [120, 4, 64] = relu(W1^T pooled^T + b1) ---
            h_sb = spool.tile([120, 4, BPC], bf16, tag="h")
            for jg in range(4):
                hp = psmm.tile([120, BPC], f32, tag="mm")
                for kc in range(8):
                    nc.tensor.matmul(hp[:], w1_sb[:, kc, 120 * jg:120 * (jg + 1)],
                                     pT_sb[:, kc, :], start=(kc == 0), stop=(kc == 7))
                nc.vector.scalar_tensor_tensor(h_sb[:, jg, :], hp[:],
                                               b1_sb[:, jg:jg + 1], zeros_sb[:],
                                               add, amax)

            # --- MLP layer 2: z^T [120, 2, 64] = W2^T h^T + b2 ---
            zT_sb = spool.tile([120, 2, BPC], bf16, tag="zT")
            for og in range(2):
                zp = psmm.tile([120, BPC], f32, tag="mm")
                for kc in range(4):
                    nc.tensor.matmul(zp[:], w2_sb[:, kc, 120 * og:120 * (og + 1)],
                                     h_sb[:, kc, :], start=(kc == 0), stop=(kc == 3))
                nc.vector.tensor_scalar_add(zT_sb[:, og, :], zp[:],
                                            b2_sb[:, og:og + 1])

            # --- local norms [1, 64] + inverses via ones-matmul ---
            lsq = scr.tile([120, 2, BPC], bf16, tag="lsq")
            nc.vector.tensor_tensor(lsq[:], zT_sb[:], zT_sb[:], mult)
            nlq_ps = psmm.tile([1, BPC], f32, tag="mm")
            nc.tensor.matmul(nlq_ps[:], ones_sb[:], lsq[:, 0, :],
                             start=True, stop=False)
            nc.tensor.matmul(nlq_ps[:], ones_sb[:], lsq[:, 1, :],
                             start=False, stop=True)
            nlr_sb = spool.tile([1, BPC], f32, tag="nlrsb")
            nc.scalar.sqrt(nlr_sb[:], nlq_ps[:])
            nlr_inv = spool.tile([1, BPC], f32, tag="nlrinv")
            nc.vector.reciprocal(nlr_inv[:], nlr_sb[:])
            nlr_bf = spool.tile([1, BPC], bf16, tag="nlrbf")
            nc.vector.tensor_copy(nlr_bf[:], nlr_inv[:])
            # local row factor with 1/T folded in: logits = s * (10/n_i) * (1/n_j)
            nlr10_bf = spool.tile([1, BPC], bf16, tag="nlr10")
            nc.vector.tensor_scalar_mul(nlr10_bf[:], nlr_inv[:], INV_T)

            # --- allgather payload [241, 64] bf16: rows 0:240 = z^T, row 240
            # = inverse row norms.  Gathering z^T directly avoids any
            # post-gather transposes and norm recomputation. ---
            zb = dram.tile([241, BPC], bf16, tag="zb")
            zg = dram.tile([241 * NCORES, BPC], bf16, tag="zg")
            nc.sync.dma_start(
                zb[0:240, :].rearrange("(og p) b -> p og b", p=120), zT_sb[:])
            nc.sync.dma_start(zb[240:241, :], nlr_bf[:])
            nc.gpsimd.collective_compute(
                "AllGather", mybir.AluOpType.bypass,
                replica_groups=[list(range(NCORES))],
                ins=[zb.opt()], outs=[zg.opt()],
            )
            zgv = zg[:].rearrange("(c r) b -> r c b", r=241)
            zfT_sb = spool.tile([120, 2, B], bf16, tag="zfT")
            for og in range(2):
                nc.sync.dma_start(
                    zfT_sb[:, og, :].rearrange("p (c b) -> p c b", b=BPC),
                    zgv[120 * og:120 * (og + 1)])
            nrow_bf = spool.tile([1, B], bf16, tag="nrowbf")
            nc.sync.dma_start(nrow_bf[:].rearrange("p (c b) -> p c b", b=BPC),
                              zgv[240:241])

            # --- inverse-norm outer product (with 1/T folded) and the
            # similarity dot [64, 512] ---
            o_ps = ps1.tile([BPC, B], f32, tag="op")
            nc.tensor.matmul(o_ps[:], nlr10_bf[:], nrow_bf[:],
                             start=True, stop=True)
            s_ps = ps1.tile([BPC, B], f32, tag="sp")
            nc.tensor.matmul(s_ps[:], zT_sb[:, 0, :], zfT_sb[:, 0, :],
                             start=True, stop=False)
            nc.tensor.matmul(s_ps[:], zT_sb[:, 1, :], zfT_sb[:, 1, :],
                             start=False, stop=True)

            # --- logits = cos/T (diag NOT masked); the diagonal's exp is
            # subtracted on the host: lse = 10 + ln(esum - ediag).  exp bias
            # is a constant -10 since the diagonal ~10 is the row max. ---
            o_sb = scr.tile([BPC, B], f32, tag="osb")
            nc.vector.tensor_copy(o_sb[:], o_ps[:])
            logits = scr.tile([BPC, B], f32, tag="logits")
            nc.vector.tensor_tensor(logits[:], s_ps[:], o_sb[:], mult)

            res_sb = spool.tile([BPC, 3], f32, tag="res")
            e_sb = scr.tile([BPC, B], f32, tag="esb")
            nc.vector.scalar_tensor_tensor(e_sb[:], logits[:], 1.0, posm_sb,
                                           mult, mult,
                                           accum_out=res_sb[:, 0:1])
            nbias = spool.tile([BPC, 1], f32, tag="nbias")
            nc.vector.memset(nbias[:], -INV_T)
            nc.scalar.activation(e_sb[:], logits[:], AF.Exp,
                                 bias=nbias[:], scale=1.0,
                                 accum_out=res_sb[:, 1:2])
            ed_sb = scr.tile([BPC, B], f32, tag="edsb")
            nc.vector.scalar_tensor_tensor(ed_sb[:], e_sb[:], 1.0, diagm_sb,
                                           mult, mult,
                                           accum_out=res_sb[:, 2:3])
            # keep the warmup collective alive: pos += 0 * wg
            nc.vector.scalar_tensor_tensor(res_sb[:, 0:1], wg_sb[:], 0.0,
                                           res_sb[:, 0:1], mult, add)

            nc.sync.dma_start(out, res_sb[:])

    nc.compile()
    return nc


def _bf16(a):
    import ml_dtypes
    return np.asarray(a, dtype=np.float32).astype(ml_dtypes.bfloat16)


def _host_inputs(x, W1c, b1c, W2c, b2c, W1a, b1a, W2a, b2a):
    x = np.ascontiguousarray(np.asarray(x, dtype=np.float32))
    # window matrix: slab i selects columns [64-2i, 128-2i); partition p
    # (batch half p//64) must hit output row 2i + p//64, so the fixed column
    # is 64 + p//64.  Values hold the mean's 1/512.
    jwin = np.zeros((128, 128), dtype=np.float32)
    jwin[np.arange(128), 64 + np.arange(128) // 64] = 1.0 / S
    in_maps = []
    for c in range(NCORES):
        rows = np.arange(BPC)
        gl = BPC * c + rows
        if c < NCORES // 2:
            w1, bb1, w2, bb2 = W1c, b1c, W2c, b2c
        else:
            w1, bb1, w2, bb2 = W1a, b1a, W2a, b2a
        w1 = np.asarray(w1, np.float32)
        w2 = np.asarray(w2, np.float32)

        cp4 = np.zeros((128, C4_W), dtype=np.float32)
        cp4[:, C4_JW:C4_JW + 128] = jwin
        diagm = np.zeros((BPC, B), dtype=np.float32)
        diagm[rows, gl] = 1.0
        cp4[0:BPC, C4_DIA:C4_DIA + B] = diagm
        posm = np.zeros((BPC, B), dtype=np.float32)
        posm[rows, (gl + B // 2) % B] = 1.0
        cp4[0:BPC, C4_POS:C4_POS + B] = posm
        cp4[0:120, C4_B1:C4_B1 + 4] = np.asarray(bb1, np.float32).reshape(4, 120).T
        cp4[0:120, C4_B2:C4_B2 + 2] = np.asarray(bb2, np.float32).reshape(2, 120).T

        cp2 = np.zeros((128, C2_W), dtype=np.float32)
        cp2[:, C2_ID:C2_ID + 128] = np.eye(128, dtype=np.float32)
        # w1 grouped: [p, k*480 + j] = W1[k*120 + p, j]
        cp2[0:120, C2_W1:C2_W1 + 8 * D1] = (
            w1.reshape(8, 120, D1).transpose(1, 0, 2).reshape(120, 8 * D1))
        cp2[0:120, C2_W2:C2_W2 + 4 * D2] = (
            w2.reshape(4, 120, D2).transpose(1, 0, 2).reshape(120, 4 * D2))

        in_maps.append({
            "xs": x[BPC * c:BPC * (c + 1)],
            "cp4": cp4,
            "cp2": _bf16(cp2),
        })
    return in_maps


def kernel(x, W1c, b1c, W2c, b2c, W1a, b1a, W2a, b2a):
    global LAST_RESULT
    trace = bool(os.environ.get("BASS_TRACE"))
    if trace:
        _install_ntff_hook()
    from concourse import bass_utils
    if trace:
        bass_utils.upload_artifacts = lambda tmpdir: "local://skipped"

    if "nc" not in _CACHE:
        _CACHE["nc"] = _build_nc()
    nc = _CACHE["nc"]

    in_maps = _host_inputs(x, W1c, b1c, W2c, b2c, W1a, b1a, W2a, b2a)
    kwargs = {}
    if trace:
        kwargs = {"trace": True, "trace_cores": TRACE_CORES}
    res = bass_utils.run_bass_kernel_spmd(
        nc, in_maps, list(range(NCORES)), **kwargs)
    LAST_RESULT = res
    r = np.concatenate([res.results[c]["nll"] for c in range(NCORES)], axis=0)
    pos, esum, ediag = r[:, 0], r[:, 1], r[:, 2]
    nll = -pos + INV_T + np.log(np.maximum(esum - ediag, 1e-30))
    return np.asarray(nll.mean(dtype=np.float64), dtype=np.float32)
